# revision 26
# baseline (speedup 1.0000x reference)
"""Trainium2 Bass kernel for a dense cross-task transformer block.

Math notes
----------
The reference "attention" has sequence length 1 on the key axis, so
softmax(scores) == 1.0 exactly and the whole q/k/score path is dead:

    mha_len1(q_in, kv_in, ...) == (kv_in @ wv.T + bv) @ wo.T + bo

which folds (on host) into a single matmul with W = wo @ wv and
b = wo @ bv + bo.  The block is then:

    verb1 = LN(verb + noun @ W1.T + c1)          (ln_v)
    verb2 = verb1 + FFN_v(verb1)
    noun1 = LN(noun + verb2 @ W2.T + c2)         (ln_n)
    noun2 = noun1 + FFN_n(noun1)
    return verb2, noun2

Device strategy
---------------
Pure data parallel over 8 cores (batch 16384 -> 2048 rows/core), weights
replicated.  Everything is feature-major ([E, batch]) so matmuls contract
along the SBUF partition dim with weights stationary.  All activations
and weights are bf16 (fp32 PSUM accumulation); LN stats run in fp32.
Both attention phases read rhs and residual straight from SBUF-resident
tiles (verb2 never round-trips through DRAM); all tensors are chunked in
512-wide column tiles so phases pipeline at chunk granularity.

Engine balancing (the attention+LN phases are the tricky part):
 - the attention bias is applied as a rank-1 matmul (ones rhs) so the
   PSUM->SBUF evacuation is a single vector add with the residual,
 - LN mean/rstd are broadcast across partitions with K=1 matmuls, then
   copied to SBUF bf16 by the scalar engine so the LN epilogue runs as
   bf16 SBUF-only vector ops (DVE 2x mode),
 - 1/std uses reciprocal_approx_fast (plain reciprocal is 3.3us),
 - each chunk's stats post-processing is emitted one chunk late (and the
   last one inside the next phase) so the PE never waits on it.
"""

import os
import numpy as np
import ml_dtypes
from contextlib import ExitStack

import concourse.bass as bass
import concourse.bacc as bacc_mod
import concourse.mybir as mybir
import concourse.tile as tile
from concourse.bass_utils import run_bass_kernel_spmd

E = 1024          # embed dim
H2 = 2048         # FFN hidden dim
B_TOTAL = 16384
NCORES = 8
B = B_TOTAL // NCORES   # 2048 rows per core
P = 128
EPS = 1e-5
CH = 512          # column chunk
NCH = B // CH     # 4
KT = E // P       # 8
MT = E // P       # 8
HT = H2 // P      # 16

F32 = mybir.dt.float32
BF16 = mybir.dt.bfloat16
AF = mybir.ActivationFunctionType
OP = mybir.AluOpType


def _build_program():
    nc = bacc_mod.Bacc("TRN2", target_bir_lowering=False)

    vT = nc.declare_dram_parameter("vT", [E, B], BF16, isOutput=False)
    nT = nc.declare_dram_parameter("nT", [E, B], BF16, isOutput=False)
    wvo1 = nc.declare_dram_parameter("wvo1", [E, E], BF16, isOutput=False)  # (wo@wv).T
    wvo2 = nc.declare_dram_parameter("wvo2", [E, E], BF16, isOutput=False)
    bvo2 = nc.declare_dram_parameter("bvo2", [1, E], BF16, isOutput=False)
    lnv = nc.declare_dram_parameter("lnv", [P, 2 * MT], F32, isOutput=False)  # g|b
    lnn = nc.declare_dram_parameter("lnn", [P, 2 * MT], F32, isOutput=False)
    w1v = nc.declare_dram_parameter("w1v", [E, H2], BF16, isOutput=False)  # fv_w1.T
    b1v = nc.declare_dram_parameter("b1v", [P, HT], F32, isOutput=False)
    w2v = nc.declare_dram_parameter("w2v", [E, H2], BF16, isOutput=False)  # m-blocked
    b2v = nc.declare_dram_parameter("b2v", [P, MT], F32, isOutput=False)
    w1n = nc.declare_dram_parameter("w1n", [E, H2], BF16, isOutput=False)
    b1n = nc.declare_dram_parameter("b1n", [P, HT], F32, isOutput=False)
    w2n = nc.declare_dram_parameter("w2n", [E, H2], BF16, isOutput=False)
    b2n = nc.declare_dram_parameter("b2n", [P, MT], F32, isOutput=False)
    verb_out = nc.declare_dram_parameter("verb_out", [E, B], BF16, isOutput=True)
    noun_out = nc.declare_dram_parameter("noun_out", [E, B], BF16, isOutput=True)

    with tile.TileContext(nc) as tc, ExitStack() as ctx:
        const = ctx.enter_context(tc.tile_pool(name="const", bufs=1))
        rhsp = ctx.enter_context(tc.tile_pool(name="rhsp", bufs=1))    # nT; later noun2 out
        resp = ctx.enter_context(tc.tile_pool(name="resp", bufs=1))    # vT; later verb2
        lnp = ctx.enter_context(tc.tile_pool(name="lnp", bufs=1))      # verb1 / noun1
        watp = ctx.enter_context(tc.tile_pool(name="watp", bufs=1))    # wvo1 / wvo2
        w1p = ctx.enter_context(tc.tile_pool(name="w1p", bufs=1))      # w1v / w1n k-tiles
        w2p = ctx.enter_context(tc.tile_pool(name="w2p", bufs=4))      # w2 m-block stream
        hp = ctx.enter_context(tc.tile_pool(name="hp", bufs=1))        # gelu hidden
        sqp = ctx.enter_context(tc.tile_pool(name="sqp", bufs=2))      # x^2
        smp = ctx.enter_context(tc.tile_pool(name="smp", bufs=2))      # LN stats vectors
        bsp = ctx.enter_context(tc.tile_pool(name="bsp", bufs=1))      # bcast SBUF copies
        mmp = ctx.enter_context(tc.tile_pool(name="mmp", bufs=4, space="PSUM"))
        stp = ctx.enter_context(tc.tile_pool(name="stp", bufs=1, space="PSUM"))
        bcp = ctx.enter_context(tc.tile_pool(name="bcp", bufs=1, space="PSUM"))

        ones_col = const.tile([P, 1], BF16, tag="ones_col", name="ones_col")
        nc.vector.memset(ones_col[:], 1.0)
        ones_row = const.tile([1, P], BF16, tag="ones_row", name="ones_row")
        nc.vector.memset(ones_row[:], 1.0)
        ones_rhs = const.tile([1, CH], BF16, tag="ones_rhs", name="ones_rhs")
        nc.vector.memset(ones_rhs[:], 1.0)
        eps_t = const.tile([1, 1], F32, tag="eps", name="eps")
        nc.vector.memset(eps_t[:], EPS)

        def load_const(dram, shape, tag, dtype=F32):
            t = const.tile(shape, dtype, tag=tag, name=tag)
            nc.sync.dma_start(out=t[:], in_=dram[:, :])
            return t

        def chunk_tiles(pool, pfx):
            return [[pool.tile([P, CH], BF16, tag=f"{pfx}{k}c{c}", name=f"{pfx}{k}c{c}")
                     for c in range(NCH)] for k in range(KT)]

        def full_tiles(pool, pfx):
            return [pool.tile([P, B], BF16, tag=f"{pfx}{k}", name=f"{pfx}{k}")
                    for k in range(KT)]

        def dma_full(tiles, dram):
            # one 512KB DMA per k-tile (4KB per-partition lines -> big packets),
            # alternating between the two HWDGE queues
            for k in range(KT):
                eng = nc.sync if k % 2 == 0 else nc.scalar
                eng.dma_start(out=tiles[k][:], in_=dram[k * P:(k + 1) * P, :])

        def load_wat(dram):
            tiles = []
            for k in range(KT):
                t = watp.tile([P, E], BF16, tag=f"wa{k}", name=f"wa{k}")
                eng = nc.scalar if k % 2 == 0 else nc.sync
                eng.dma_start(out=t[:], in_=dram[k * P:(k + 1) * P, :])
                tiles.append(t)
            return tiles

        def load_w1(dram):
            tiles = []
            for k in range(KT):
                t = w1p.tile([P, H2], BF16, tag=f"w1_{k}", name=f"w1_{k}")
                eng = nc.scalar if k % 2 == 0 else nc.sync
                eng.dma_start(out=t[:], in_=dram[k * P:(k + 1) * P, :])
                tiles.append(t)
            return tiles

        def attn_phase(wt, rhs, resid, outt, bias_row, gb_pb, k_outer_c0=False,
                       boundary_filler=None):
            """outt[m][c] (bf16) = LN(resid + attn-matmul + bias).

            Returns a flush() that emits the final chunk's LN epilogue;
            the caller interleaves it into the next phase's first MMs.
            """
            g_pb = gb_pb[:, 0:MT]
            b_pb = gb_pb[:, MT:2 * MT]

            def emit_post(c, sx, sqs):
                nm = smp.tile([1, CH], BF16, tag="nm", name="nm")
                nc.scalar.activation(nm[:], sx[:], AF.Copy, scale=-1.0 / E)
                t1 = smp.tile([1, CH], F32, tag="t1", name="t1")
                nc.scalar.activation(t1[:], sqs[:], AF.Copy, scale=1.0 / E)
                m2 = smp.tile([1, CH], F32, tag="m2", name="m2")
                nc.scalar.activation(m2[:], nm[:], AF.Square)
                nc.vector.tensor_sub(t1[:], t1[:], m2[:])          # var
                nc.scalar.activation(t1[:], t1[:], AF.Sqrt, bias=eps_t[:])
                rs = smp.tile([1, CH], F32, tag="rs", name="rs")
                nc.vector.reciprocal_approx_fast(out=rs[:], in_=t1[:])
                rsb = smp.tile([1, CH], BF16, tag="rsb", name="rsb")
                nc.scalar.activation(rsb[:], rs[:], AF.Copy)
                nmB = bcp.tile([P, CH], F32, tag="nmB", name="nmB")
                nc.tensor.matmul(nmB[:], lhsT=ones_row[:], rhs=nm[:],
                                 start=True, stop=True)
                rsB = bcp.tile([P, CH], F32, tag="rsB", name="rsB")
                nc.tensor.matmul(rsB[:], lhsT=ones_row[:], rhs=rsb[:],
                                 start=True, stop=True)
                nmS = bsp.tile([P, CH], BF16, tag="nmS", name="nmS")
                nc.scalar.activation(nmS[:], nmB[:], AF.Copy)
                rsS = bsp.tile([P, CH], BF16, tag="rsS", name="rsS")
                nc.scalar.activation(rsS[:], rsB[:], AF.Copy)
                for m in range(MT):
                    t = outt[m][c]
                    nc.vector.tensor_add(t[:], t[:], nmS[:])
                    nc.vector.tensor_mul(t[:], t[:], rsS[:])
                    nc.vector.tensor_scalar(
                        t[:], t[:], g_pb[:, m:m + 1], b_pb[:, m:m + 1],
                        OP.mult, OP.add)

            def epilog_m(c, m, ps, sx, sqs):
                cs = slice(c * CH, (c + 1) * CH)
                if bias_row is not None:
                    nc.tensor.matmul(
                        ps[:], lhsT=bias_row[:, m * P:(m + 1) * P],
                        rhs=ones_rhs[:], start=False, stop=True)
                xt = outt[m][c]
                nc.vector.tensor_add(xt[:], ps[:], resid[m][:, cs])
                sq = sqp.tile([P, CH], BF16, tag="sq", name="sq")
                nc.scalar.activation(sq[:], xt[:], AF.Square)
                nc.tensor.matmul(sx[:], lhsT=ones_col[:], rhs=xt[:],
                                 start=(m == 0), stop=(m == MT - 1))
                nc.tensor.matmul(sqs[:], lhsT=ones_col[:], rhs=sq[:],
                                 start=(m == 0), stop=(m == MT - 1))

            last_stop = bias_row is None
            pend = None
            for c in range(NCH):
                sx = stp.tile([1, CH], F32, tag="sx", name="sx")
                sqs = stp.tile([1, CH], F32, tag="sqs", name="sqs")
                cs = slice(c * CH, (c + 1) * CH)
                if c == 0 and k_outer_c0:
                    # k-outer over m-quads: start computing as soon as the
                    # first (weight, rhs) k-tile pair lands from DRAM
                    for half in (range(0, 4), range(4, 8)):
                        pss = {m: mmp.tile([P, CH], F32, tag="mm", name="mm")
                               for m in half}
                        for k in range(KT):
                            for m in half:
                                nc.tensor.matmul(
                                    pss[m][:], lhsT=wt[k][:, m * P:(m + 1) * P],
                                    rhs=rhs[k][:, cs],
                                    start=(k == 0),
                                    stop=(k == KT - 1 and last_stop))
                        for m in half:
                            epilog_m(c, m, pss[m], sx, sqs)
                else:
                    for m in range(MT):
                        ps = mmp.tile([P, CH], F32, tag="mm", name="mm")
                        for k in range(KT):
                            nc.tensor.matmul(
                                ps[:], lhsT=wt[k][:, m * P:(m + 1) * P],
                                rhs=rhs[k][:, cs],
                                start=(k == 0),
                                stop=(k == KT - 1 and last_stop))
                        epilog_m(c, m, ps, sx, sqs)
                        if m == MT - 2 and pend is not None:
                            emit_post(*pend)
                            pend = None
                pend = (c, sx, sqs)
                if c == 2 and boundary_filler is not None:
                    boundary_filler()   # independent PE work over the boundary
            return lambda: emit_post(*pend)

        def ffn_mm1_group(in_t, w1_t, b1_pb, ns, hm):
            ps = mmp.tile([P, CH], F32, tag="mm", name="mm")
            for k in range(KT):
                nc.tensor.matmul(
                    ps[:], lhsT=w1_t[k][:, hm * P:(hm + 1) * P],
                    rhs=in_t[k][ns][:],
                    start=(k == 0), stop=(k == KT - 1))
            h = hp.tile([P, CH], BF16, tag=f"h{hm}", name=f"h{hm}")
            nc.scalar.activation(h[:], ps[:], AF.Gelu,
                                 bias=b1_pb[:, hm:hm + 1])
            return h

        def ffn_phase(in_t, w1_t, w2_dram, b1_pb, b2_pb, outt, out_dram,
                      pre_flush=None, hoisted=None):
            """outt[m][ns] (bf16) = in + W2.T@gelu(W1.T@in + b1) + b2."""
            for ns in range(NCH):
                hl = []
                first_computed = True
                for hm in range(HT):
                    if ns == 0 and hoisted is not None and hm in hoisted:
                        hl.append(hoisted[hm])
                        continue
                    h = ffn_mm1_group(in_t, w1_t, b1_pb, ns, hm)
                    hl.append(h)
                    if ns == 0 and first_computed and pre_flush is not None:
                        pre_flush()   # previous phase's last LN epilogue
                        pre_flush = None
                    first_computed = False
                for m in range(MT):
                    wb = w2p.tile([P, H2], BF16, tag="w2s", name="w2s")
                    eng = nc.scalar if m % 2 == 0 else nc.sync
                    eng.dma_start(out=wb[:], in_=w2_dram[m * P:(m + 1) * P, :])
                    ps = mmp.tile([P, CH], F32, tag="mm", name="mm")
                    for hm in range(HT):
                        nc.tensor.matmul(
                            ps[:], lhsT=wb[:, hm * P:(hm + 1) * P],
                            rhs=hl[hm][:],
                            start=(hm == 0), stop=(hm == HT - 1))
                    nc.vector.tensor_scalar(
                        ps[:], ps[:], b2_pb[:, m:m + 1], None, OP.add)
                    ot = outt[m][:, ns * CH:(ns + 1) * CH]
                    nc.vector.tensor_add(ot, ps[:], in_t[m][ns][:])
                    (nc.sync if m % 2 == 0 else nc.scalar).dma_start(
                        out=out_dram[m * P:(m + 1) * P, ns * CH:(ns + 1) * CH],
                        in_=ot)

        _REP = int(os.environ.get("BENCH_REPEAT", "1"))
        for _rep in range(_REP):
            # ---- phase A: verb attends to noun, LN -> verb1 ----
            nt_t = full_tiles(rhsp, "n")         # noun rhs (and phase-C residual)
            wa1 = []
            for k in range(KT):
                t = watp.tile([P, E], BF16, tag=f"wa{k}", name=f"wa{k}")
                (nc.scalar if k % 2 == 0 else nc.sync).dma_start(
                    out=t[:], in_=wvo1[k * P:(k + 1) * P, :])
                wa1.append(t)
                (nc.sync if k % 2 == 0 else nc.scalar).dma_start(
                    out=nt_t[k][:], in_=nT[k * P:(k + 1) * P, :])
            bvo2_r = load_const(bvo2, [1, E], "bvo2", BF16)
            lnv_pb = load_const(lnv, [P, 2 * MT], "lnv")
            lnn_pb = load_const(lnn, [P, 2 * MT], "lnn")
            b1v_pb = load_const(b1v, [P, HT], "b1v")
            b2v_pb = load_const(b2v, [P, MT], "b2v")
            b1n_pb = load_const(b1n, [P, HT], "b1n")
            b2n_pb = load_const(b2n, [P, MT], "b2n")
            vt_t = full_tiles(resp, "v")         # verb residual (bvo1 pre-added)
            for k in range(KT):
                eng = nc.sync if k % 2 == 0 else nc.scalar
                eng.dma_start(out=vt_t[k][:, 0:CH], in_=vT[k * P:(k + 1) * P, 0:CH])
            for k in range(KT):
                eng = nc.sync if k % 2 == 0 else nc.scalar
                eng.dma_start(out=vt_t[k][:, CH:B], in_=vT[k * P:(k + 1) * P, CH:B])
            w1v_t = load_w1(w1v)                 # prefetch for phase B
            verb1 = chunk_tiles(lnp, "l")
            hoist_b = {}

            def fill_b():
                for hm in range(4):
                    hoist_b[hm] = ffn_mm1_group(verb1, w1v_t, b1v_pb, 0, hm)

            fl_a = attn_phase(wa1, nt_t, vt_t, verb1, None, lnv_pb,
                              k_outer_c0=True, boundary_filler=fill_b)

            # ---- phase B: verb FFN -> verb2 (written into the vT tiles) ----
            wa2 = load_wat(wvo2)                 # prefetch for phase C
            ffn_phase(verb1, w1v_t, w2v, b1v_pb, b2v_pb, vt_t, verb_out,
                      pre_flush=fl_a, hoisted=hoist_b)

            # ---- phase C: noun attends to verb2, LN -> noun1 ----
            w1n_t = load_w1(w1n)                 # prefetch for phase D
            noun1 = chunk_tiles(lnp, "l")
            hoist_d = {}

            def fill_d():
                for hm in range(4):
                    hoist_d[hm] = ffn_mm1_group(noun1, w1n_t, b1n_pb, 0, hm)

            fl_c = attn_phase(wa2, vt_t, nt_t, noun1, bvo2_r, lnn_pb,
                              boundary_filler=fill_d)

            # ---- phase D: noun FFN -> noun2 (written into the nT tiles) ----
            ffn_phase(noun1, w1n_t, w2n, b1n_pb, b2n_pb, nt_t, noun_out,
                      pre_flush=fl_c, hoisted=hoist_d)

    nc.finalize()
    return nc


_prog_cache = {}


def _get_program():
    if "nc" not in _prog_cache:
        _prog_cache["nc"] = _build_program()
    return _prog_cache["nc"]


def _pvec(v, ntiles):
    # [ntiles*128] -> [128, ntiles] with (p, t) = v[t*128+p]
    return np.ascontiguousarray(np.asarray(v, np.float32).reshape(ntiles, P).T)


def _prepare_maps(inputs):
    f32 = np.float32
    bf16 = ml_dtypes.bfloat16
    g = {k: np.asarray(v, f32) for k, v in inputs.items()}

    def fold(p):
        w = g[f"{p}_wo"] @ g[f"{p}_wv"]
        b = g[f"{p}_wo"] @ g[f"{p}_bv"] + g[f"{p}_bo"]
        return (np.ascontiguousarray(w.T).astype(bf16),
                np.ascontiguousarray(b.reshape(1, E)).astype(bf16))

    def w2block(w2):
        # [m*128+p, hm*128+c] = w2T[hm*128+p, m*128+c]
        w2T = np.ascontiguousarray(w2.T)  # [H2, E]
        r = w2T.reshape(HT, P, MT, P).transpose(2, 1, 0, 3).reshape(E, H2)
        return np.ascontiguousarray(r).astype(bf16)

    wvo1, bvo1 = fold("v2n")
    wvo2, bvo2 = fold("n2v")
    common = {
        "wvo1": wvo1, "wvo2": wvo2, "bvo2": bvo2,
        "lnv": np.concatenate([_pvec(g["ln_v_g"], MT), _pvec(g["ln_v_b"], MT)], axis=1),
        "lnn": np.concatenate([_pvec(g["ln_n_g"], MT), _pvec(g["ln_n_b"], MT)], axis=1),
        "w1v": np.ascontiguousarray(g["fv_w1"].T).astype(bf16),
        "b1v": _pvec(g["fv_b1"], HT),
        "w2v": w2block(g["fv_w2"]), "b2v": _pvec(g["fv_b2"], MT),
        "w1n": np.ascontiguousarray(g["fn_w1"].T).astype(bf16),
        "b1n": _pvec(g["fn_b1"], HT),
        "w2n": w2block(g["fn_w2"]), "b2n": _pvec(g["fn_b2"], MT),
    }
    # verb residual with the folded v2n attention bias pre-added
    vT = np.ascontiguousarray(
        g["verb_features"].T + bvo1.astype(f32).reshape(E, 1)).astype(bf16)
    nT = np.ascontiguousarray(g["noun_features"].T).astype(bf16)
    in_maps = []
    for i in range(NCORES):
        cs = slice(i * B, (i + 1) * B)
        m = dict(common)
        m["vT"] = np.ascontiguousarray(vT[:, cs])
        m["nT"] = np.ascontiguousarray(nT[:, cs])
        in_maps.append(m)
    return in_maps


def kernel(**inputs):
    nc = _get_program()
    in_maps = _prepare_maps(inputs)
    res = run_bass_kernel_spmd(nc, in_maps, list(range(NCORES))).results
    verb = np.concatenate(
        [res[i]["verb_out"].astype(np.float32) for i in range(NCORES)], axis=1)
    noun = np.concatenate(
        [res[i]["noun_out"].astype(np.float32) for i in range(NCORES)], axis=1)
    return np.ascontiguousarray(verb.T), np.ascontiguousarray(noun.T)


# revision 27
# speedup vs baseline: 1.1797x; 1.1797x over previous
"""Trainium2 Bass kernel for a dense cross-task transformer block.

Math notes
----------
The reference "attention" has sequence length 1 on the key axis, so
softmax(scores) == 1.0 exactly and the whole q/k/score path is dead:

    mha_len1(q_in, kv_in, ...) == (kv_in @ wv.T + bv) @ wo.T + bo

which folds (on host) into a single matmul with W = wo @ wv and
b = wo @ bv + bo.  The block is then:

    verb1 = LN(verb + noun @ W1.T + c1)          (ln_v)
    verb2 = verb1 + FFN_v(verb1)
    noun1 = LN(noun + verb2 @ W2.T + c2)         (ln_n)
    noun2 = noun1 + FFN_n(noun1)
    return verb2, noun2

Device strategy
---------------
Pure data parallel over 8 cores (batch 16384 -> 2048 rows/core), weights
replicated.  Everything is feature-major ([E, batch]) so matmuls contract
along the SBUF partition dim with weights stationary.  All activations
and weights are bf16 (fp32 PSUM accumulation); LN stats run in fp32.
Both attention phases read rhs and residual straight from SBUF-resident
tiles (verb2 never round-trips through DRAM); all tensors are chunked in
512-wide column tiles so phases pipeline at chunk granularity.

Engine balancing (the attention+LN phases are the tricky part):
 - the attention bias is applied as a rank-1 matmul (ones rhs) so the
   PSUM->SBUF evacuation is a single vector add with the residual,
 - LN mean/rstd are broadcast across partitions with K=1 matmuls, then
   copied to SBUF bf16 by the scalar engine so the LN epilogue runs as
   bf16 SBUF-only vector ops (DVE 2x mode),
 - 1/std uses reciprocal_approx_fast (plain reciprocal is 3.3us),
 - each chunk's stats post-processing is emitted one chunk late (and the
   last one inside the next phase) so the PE never waits on it.
"""

import os
import numpy as np
import ml_dtypes
from contextlib import ExitStack

import concourse.bass as bass
import concourse.bacc as bacc_mod
import concourse.mybir as mybir
import concourse.tile as tile
from concourse.bass_utils import run_bass_kernel_spmd

E = 1024          # embed dim
H2 = 2048         # FFN hidden dim
B_TOTAL = 16384
NCORES = 8
B = B_TOTAL // NCORES   # 2048 rows per core
P = 128
EPS = 1e-5
CH = 512          # column chunk
NCH = B // CH     # 4
KT = E // P       # 8
MT = E // P       # 8
HT = H2 // P      # 16

F32 = mybir.dt.float32
BF16 = mybir.dt.bfloat16
AF = mybir.ActivationFunctionType
OP = mybir.AluOpType


def _build_program():
    nc = bacc_mod.Bacc("TRN2", target_bir_lowering=False)

    vT = nc.declare_dram_parameter("vT", [E, B], BF16, isOutput=False)
    nT = nc.declare_dram_parameter("nT", [E, B], BF16, isOutput=False)
    wvo1 = nc.declare_dram_parameter("wvo1", [E, E], BF16, isOutput=False)  # (wo@wv).T
    wvo2 = nc.declare_dram_parameter("wvo2", [E, E], BF16, isOutput=False)
    bvo2 = nc.declare_dram_parameter("bvo2", [1, E], BF16, isOutput=False)
    lnv = nc.declare_dram_parameter("lnv", [P, 2 * MT], F32, isOutput=False)  # g|b
    lnn = nc.declare_dram_parameter("lnn", [P, 2 * MT], F32, isOutput=False)
    w1v = nc.declare_dram_parameter("w1v", [E, H2], BF16, isOutput=False)  # fv_w1.T
    b1v = nc.declare_dram_parameter("b1v", [P, HT], F32, isOutput=False)
    w2v = nc.declare_dram_parameter("w2v", [E, H2], BF16, isOutput=False)  # m-blocked
    b2v = nc.declare_dram_parameter("b2v", [P, MT], F32, isOutput=False)
    w1n = nc.declare_dram_parameter("w1n", [E, H2], BF16, isOutput=False)
    b1n = nc.declare_dram_parameter("b1n", [P, HT], F32, isOutput=False)
    w2n = nc.declare_dram_parameter("w2n", [E, H2], BF16, isOutput=False)
    b2n = nc.declare_dram_parameter("b2n", [P, MT], F32, isOutput=False)
    verb_out = nc.declare_dram_parameter("verb_out", [E, B], BF16, isOutput=True)
    noun_out = nc.declare_dram_parameter("noun_out", [E, B], BF16, isOutput=True)

    with tile.TileContext(nc) as tc, ExitStack() as ctx:
        const = ctx.enter_context(tc.tile_pool(name="const", bufs=1))
        rhsp = ctx.enter_context(tc.tile_pool(name="rhsp", bufs=1))    # nT; later noun2 out
        resp = ctx.enter_context(tc.tile_pool(name="resp", bufs=1))    # vT; later verb2
        lnp = ctx.enter_context(tc.tile_pool(name="lnp", bufs=1))      # verb1 / noun1
        watp = ctx.enter_context(tc.tile_pool(name="watp", bufs=1))    # wvo1 / wvo2
        w1p = ctx.enter_context(tc.tile_pool(name="w1p", bufs=1))      # w1v / w1n k-tiles
        w2p = ctx.enter_context(tc.tile_pool(name="w2p", bufs=4))      # w2 m-block stream
        hp = ctx.enter_context(tc.tile_pool(name="hp", bufs=1))        # gelu hidden
        sqp = ctx.enter_context(tc.tile_pool(name="sqp", bufs=2))      # x^2
        smp = ctx.enter_context(tc.tile_pool(name="smp", bufs=2))      # LN stats vectors
        bsp = ctx.enter_context(tc.tile_pool(name="bsp", bufs=1))      # bcast SBUF copies
        mmp = ctx.enter_context(tc.tile_pool(name="mmp", bufs=4, space="PSUM"))
        stp = ctx.enter_context(tc.tile_pool(name="stp", bufs=1, space="PSUM"))
        bcp = ctx.enter_context(tc.tile_pool(name="bcp", bufs=1, space="PSUM"))

        ones_col = const.tile([P, 1], BF16, tag="ones_col", name="ones_col")
        nc.vector.memset(ones_col[:], 1.0)
        ones_row = const.tile([1, P], BF16, tag="ones_row", name="ones_row")
        nc.vector.memset(ones_row[:], 1.0)
        ones_rhs = const.tile([1, CH], BF16, tag="ones_rhs", name="ones_rhs")
        nc.vector.memset(ones_rhs[:], 1.0)
        eps_t = const.tile([1, 1], F32, tag="eps", name="eps")
        nc.vector.memset(eps_t[:], EPS)

        def load_const(dram, shape, tag, dtype=F32):
            t = const.tile(shape, dtype, tag=tag, name=tag)
            nc.sync.dma_start(out=t[:], in_=dram[:, :])
            return t

        def chunk_tiles(pool, pfx):
            return [[pool.tile([P, CH], BF16, tag=f"{pfx}{k}c{c}", name=f"{pfx}{k}c{c}")
                     for c in range(NCH)] for k in range(KT)]

        def full_tiles(pool, pfx):
            return [pool.tile([P, B], BF16, tag=f"{pfx}{k}", name=f"{pfx}{k}")
                    for k in range(KT)]

        def dma_full(tiles, dram):
            # one 512KB DMA per k-tile (4KB per-partition lines -> big packets),
            # alternating between the two HWDGE queues
            for k in range(KT):
                eng = nc.sync if k % 2 == 0 else nc.scalar
                eng.dma_start(out=tiles[k][:], in_=dram[k * P:(k + 1) * P, :])

        def load_wat(dram):
            tiles = []
            for k in range(KT):
                t = watp.tile([P, E], BF16, tag=f"wa{k}", name=f"wa{k}")
                eng = nc.scalar if k % 2 == 0 else nc.sync
                eng.dma_start(out=t[:], in_=dram[k * P:(k + 1) * P, :])
                tiles.append(t)
            return tiles

        def load_w1(dram):
            tiles = []
            for k in range(KT):
                t = w1p.tile([P, H2], BF16, tag=f"w1_{k}", name=f"w1_{k}")
                eng = nc.scalar if k % 2 == 0 else nc.sync
                eng.dma_start(out=t[:], in_=dram[k * P:(k + 1) * P, :])
                tiles.append(t)
            return tiles

        def attn_phase(wt, rhs, resid, outt, bias_row, gb_pb, k_outer_c0=False):
            """outt[m][c] (bf16) = LN(resid + attn-matmul + bias).

            Returns a flush() that emits the final chunk's LN epilogue;
            the caller interleaves it into the next phase's first MMs.
            """
            g_pb = gb_pb[:, 0:MT]
            b_pb = gb_pb[:, MT:2 * MT]

            def emit_post(c, sx, sqs):
                nm = smp.tile([1, CH], BF16, tag="nm", name="nm")
                nc.scalar.activation(nm[:], sx[:], AF.Copy, scale=-1.0 / E)
                t1 = smp.tile([1, CH], F32, tag="t1", name="t1")
                nc.scalar.activation(t1[:], sqs[:], AF.Copy, scale=1.0 / E)
                m2 = smp.tile([1, CH], F32, tag="m2", name="m2")
                nc.scalar.activation(m2[:], nm[:], AF.Square)
                nc.vector.tensor_sub(t1[:], t1[:], m2[:])          # var
                nc.scalar.activation(t1[:], t1[:], AF.Sqrt, bias=eps_t[:])
                rs = smp.tile([1, CH], F32, tag="rs", name="rs")
                nc.vector.reciprocal_approx_fast(out=rs[:], in_=t1[:])
                rsb = smp.tile([1, CH], BF16, tag="rsb", name="rsb")
                nc.scalar.activation(rsb[:], rs[:], AF.Copy)
                nmB = bcp.tile([P, CH], F32, tag="nmB", name="nmB")
                nc.tensor.matmul(nmB[:], lhsT=ones_row[:], rhs=nm[:],
                                 start=True, stop=True)
                rsB = bcp.tile([P, CH], F32, tag="rsB", name="rsB")
                nc.tensor.matmul(rsB[:], lhsT=ones_row[:], rhs=rsb[:],
                                 start=True, stop=True)
                nmS = bsp.tile([P, CH], BF16, tag="nmS", name="nmS")
                nc.scalar.activation(nmS[:], nmB[:], AF.Copy)
                rsS = bsp.tile([P, CH], BF16, tag="rsS", name="rsS")
                nc.scalar.activation(rsS[:], rsB[:], AF.Copy)
                for m in range(MT):
                    t = outt[m][c]
                    nc.vector.tensor_add(t[:], t[:], nmS[:])
                    nc.vector.tensor_mul(t[:], t[:], rsS[:])
                    nc.vector.tensor_scalar(
                        t[:], t[:], g_pb[:, m:m + 1], b_pb[:, m:m + 1],
                        OP.mult, OP.add)

            def epilog_m(c, m, ps, sx, sqs):
                cs = slice(c * CH, (c + 1) * CH)
                if bias_row is not None:
                    nc.tensor.matmul(
                        ps[:], lhsT=bias_row[:, m * P:(m + 1) * P],
                        rhs=ones_rhs[:], start=False, stop=True)
                xt = outt[m][c]
                nc.vector.tensor_add(xt[:], ps[:], resid[m][:, cs])
                sq = sqp.tile([P, CH], BF16, tag="sq", name="sq")
                nc.scalar.activation(sq[:], xt[:], AF.Square)
                nc.tensor.matmul(sx[:], lhsT=ones_col[:], rhs=xt[:],
                                 start=(m == 0), stop=(m == MT - 1))
                nc.tensor.matmul(sqs[:], lhsT=ones_col[:], rhs=sq[:],
                                 start=(m == 0), stop=(m == MT - 1))

            last_stop = bias_row is None
            pend = None
            for c in range(NCH):
                sx = stp.tile([1, CH], F32, tag="sx", name="sx")
                sqs = stp.tile([1, CH], F32, tag="sqs", name="sqs")
                cs = slice(c * CH, (c + 1) * CH)
                if c == 0 and k_outer_c0:
                    # k-outer over m-quads: start computing as soon as the
                    # first (weight, rhs) k-tile pair lands from DRAM
                    for half in (range(0, 4), range(4, 8)):
                        pss = {m: mmp.tile([P, CH], F32, tag="mm", name="mm")
                               for m in half}
                        for k in range(KT):
                            for m in half:
                                nc.tensor.matmul(
                                    pss[m][:], lhsT=wt[k][:, m * P:(m + 1) * P],
                                    rhs=rhs[k][:, cs],
                                    start=(k == 0),
                                    stop=(k == KT - 1 and last_stop))
                        for m in half:
                            epilog_m(c, m, pss[m], sx, sqs)
                else:
                    for m in range(MT):
                        ps = mmp.tile([P, CH], F32, tag="mm", name="mm")
                        for k in range(KT):
                            nc.tensor.matmul(
                                ps[:], lhsT=wt[k][:, m * P:(m + 1) * P],
                                rhs=rhs[k][:, cs],
                                start=(k == 0),
                                stop=(k == KT - 1 and last_stop))
                        epilog_m(c, m, ps, sx, sqs)
                        if m == MT - 2 and pend is not None:
                            emit_post(*pend)
                            pend = None
                pend = (c, sx, sqs)
            return lambda: emit_post(*pend)

        def ffn_phase(in_t, w1_t, w2_dram, b1_pb, b2_pb, outt, out_dram,
                      pre_flush=None):
            """outt[m][ns] (bf16) = in + W2.T@gelu(W1.T@in + b1) + b2."""
            for ns in range(NCH):
                hl = []
                for hm in range(HT):
                    ps = mmp.tile([P, CH], F32, tag="mm", name="mm")
                    for k in range(KT):
                        nc.tensor.matmul(
                            ps[:], lhsT=w1_t[k][:, hm * P:(hm + 1) * P],
                            rhs=in_t[k][ns][:],
                            start=(k == 0), stop=(k == KT - 1))
                    h = hp.tile([P, CH], BF16, tag=f"h{hm}", name=f"h{hm}")
                    nc.scalar.activation(h[:], ps[:], AF.Gelu,
                                         bias=b1_pb[:, hm:hm + 1])
                    hl.append(h)
                    if ns == 0 and hm == 1 and pre_flush is not None:
                        pre_flush()   # previous phase's last LN epilogue
                        pre_flush = None
                for m in range(MT):
                    wb = w2p.tile([P, H2], BF16, tag="w2s", name="w2s")
                    eng = nc.scalar if m % 2 == 0 else nc.sync
                    eng.dma_start(out=wb[:], in_=w2_dram[m * P:(m + 1) * P, :])
                    ps = mmp.tile([P, CH], F32, tag="mm", name="mm")
                    for hm in range(HT):
                        nc.tensor.matmul(
                            ps[:], lhsT=wb[:, hm * P:(hm + 1) * P],
                            rhs=hl[hm][:],
                            start=(hm == 0), stop=(hm == HT - 1))
                    nc.vector.tensor_scalar(
                        ps[:], ps[:], b2_pb[:, m:m + 1], None, OP.add)
                    ot = outt[m][:, ns * CH:(ns + 1) * CH]
                    nc.vector.tensor_add(ot, ps[:], in_t[m][ns][:])
                    (nc.sync if m % 2 == 0 else nc.scalar).dma_start(
                        out=out_dram[m * P:(m + 1) * P, ns * CH:(ns + 1) * CH],
                        in_=ot)

        _REP = int(os.environ.get("BENCH_REPEAT", "1"))
        for _rep in range(_REP):
            # ---- phase A: verb attends to noun, LN -> verb1 ----
            nt_t = full_tiles(rhsp, "n")         # noun rhs (and phase-C residual)
            wa1 = []
            for k in range(KT):
                t = watp.tile([P, E], BF16, tag=f"wa{k}", name=f"wa{k}")
                (nc.scalar if k % 2 == 0 else nc.sync).dma_start(
                    out=t[:], in_=wvo1[k * P:(k + 1) * P, :])
                wa1.append(t)
                (nc.sync if k % 2 == 0 else nc.scalar).dma_start(
                    out=nt_t[k][:], in_=nT[k * P:(k + 1) * P, :])
            bvo2_r = load_const(bvo2, [1, E], "bvo2", BF16)
            lnv_pb = load_const(lnv, [P, 2 * MT], "lnv")
            lnn_pb = load_const(lnn, [P, 2 * MT], "lnn")
            b1v_pb = load_const(b1v, [P, HT], "b1v")
            b2v_pb = load_const(b2v, [P, MT], "b2v")
            b1n_pb = load_const(b1n, [P, HT], "b1n")
            b2n_pb = load_const(b2n, [P, MT], "b2n")
            vt_t = full_tiles(resp, "v")         # verb residual (bvo1 pre-added)
            for k in range(KT):
                eng = nc.sync if k % 2 == 0 else nc.scalar
                eng.dma_start(out=vt_t[k][:, 0:CH], in_=vT[k * P:(k + 1) * P, 0:CH])
            for k in range(KT):
                eng = nc.sync if k % 2 == 0 else nc.scalar
                eng.dma_start(out=vt_t[k][:, CH:B], in_=vT[k * P:(k + 1) * P, CH:B])
            w1v_t = load_w1(w1v)                 # prefetch for phase B
            verb1 = chunk_tiles(lnp, "l")
            fl_a = attn_phase(wa1, nt_t, vt_t, verb1, None, lnv_pb,
                              k_outer_c0=True)

            # ---- phase B: verb FFN -> verb2 (written into the vT tiles) ----
            wa2 = load_wat(wvo2)                 # prefetch for phase C
            ffn_phase(verb1, w1v_t, w2v, b1v_pb, b2v_pb, vt_t, verb_out,
                      pre_flush=fl_a)

            # ---- phase C: noun attends to verb2, LN -> noun1 ----
            w1n_t = load_w1(w1n)                 # prefetch for phase D
            noun1 = chunk_tiles(lnp, "l")
            fl_c = attn_phase(wa2, vt_t, nt_t, noun1, bvo2_r, lnn_pb)

            # ---- phase D: noun FFN -> noun2 (written into the nT tiles) ----
            ffn_phase(noun1, w1n_t, w2n, b1n_pb, b2n_pb, nt_t, noun_out,
                      pre_flush=fl_c)

    nc.finalize()
    return nc


_prog_cache = {}


def _get_program():
    if "nc" not in _prog_cache:
        _prog_cache["nc"] = _build_program()
    return _prog_cache["nc"]


def _pvec(v, ntiles):
    # [ntiles*128] -> [128, ntiles] with (p, t) = v[t*128+p]
    return np.ascontiguousarray(np.asarray(v, np.float32).reshape(ntiles, P).T)


def _prepare_maps(inputs):
    f32 = np.float32
    bf16 = ml_dtypes.bfloat16
    g = {k: np.asarray(v, f32) for k, v in inputs.items()}

    def fold(p):
        w = g[f"{p}_wo"] @ g[f"{p}_wv"]
        b = g[f"{p}_wo"] @ g[f"{p}_bv"] + g[f"{p}_bo"]
        return (np.ascontiguousarray(w.T).astype(bf16),
                np.ascontiguousarray(b.reshape(1, E)).astype(bf16))

    def w2block(w2):
        # [m*128+p, hm*128+c] = w2T[hm*128+p, m*128+c]
        w2T = np.ascontiguousarray(w2.T)  # [H2, E]
        r = w2T.reshape(HT, P, MT, P).transpose(2, 1, 0, 3).reshape(E, H2)
        return np.ascontiguousarray(r).astype(bf16)

    wvo1, bvo1 = fold("v2n")
    wvo2, bvo2 = fold("n2v")
    common = {
        "wvo1": wvo1, "wvo2": wvo2, "bvo2": bvo2,
        "lnv": np.concatenate([_pvec(g["ln_v_g"], MT), _pvec(g["ln_v_b"], MT)], axis=1),
        "lnn": np.concatenate([_pvec(g["ln_n_g"], MT), _pvec(g["ln_n_b"], MT)], axis=1),
        "w1v": np.ascontiguousarray(g["fv_w1"].T).astype(bf16),
        "b1v": _pvec(g["fv_b1"], HT),
        "w2v": w2block(g["fv_w2"]), "b2v": _pvec(g["fv_b2"], MT),
        "w1n": np.ascontiguousarray(g["fn_w1"].T).astype(bf16),
        "b1n": _pvec(g["fn_b1"], HT),
        "w2n": w2block(g["fn_w2"]), "b2n": _pvec(g["fn_b2"], MT),
    }
    # verb residual with the folded v2n attention bias pre-added
    vT = np.ascontiguousarray(
        g["verb_features"].T + bvo1.astype(f32).reshape(E, 1)).astype(bf16)
    nT = np.ascontiguousarray(g["noun_features"].T).astype(bf16)
    in_maps = []
    for i in range(NCORES):
        cs = slice(i * B, (i + 1) * B)
        m = dict(common)
        m["vT"] = np.ascontiguousarray(vT[:, cs])
        m["nT"] = np.ascontiguousarray(nT[:, cs])
        in_maps.append(m)
    return in_maps


def kernel(**inputs):
    nc = _get_program()
    in_maps = _prepare_maps(inputs)
    res = run_bass_kernel_spmd(nc, in_maps, list(range(NCORES))).results
    verb = np.concatenate(
        [res[i]["verb_out"].astype(np.float32) for i in range(NCORES)], axis=1)
    noun = np.concatenate(
        [res[i]["noun_out"].astype(np.float32) for i in range(NCORES)], axis=1)
    return np.ascontiguousarray(verb.T), np.ascontiguousarray(noun.T)
